# revision 86
# baseline (speedup 1.0000x reference)
"""Trainium2 Bass kernel for MiniKDALayer (chunked delta-rule + gated FFN).

Sequence-parallel over 8 cores (T=8192 -> 1024 rows/core):
  L1 kernel: x^T transpose, fused projections, PoPE, per-chunk (C=64)
     delta-rule quantities via the WY chunked form -> per-chunk affine state
     maps (A^T, B) and chunk-local output pieces (Obase^T, Qeff^T).
  Host: 128-step chunk-state scan over tiny (32,16) states.
  L2 kernel: o assembly, post-RMSNorm, gating, Wout+residual, FFN with
     fp32r matmuls, final residual, straight (t-major) output.
"""
import math

import numpy as np

import concourse.bass as bass
import concourse.bacc as bacc
import concourse.mybir as mybir
import concourse.tile as tile
from concourse.bass_utils import run_bass_kernel_spmd

F32 = mybir.dt.float32
F32R = mybir.dt.float32r
BF16 = mybir.dt.bfloat16
F8 = mybir.dt.float8e4
DR = mybir.MatmulPerfMode.DoubleRow
AF = mybir.ActivationFunctionType
OP = mybir.AluOpType

T, D, DK, DKP, DV = 8192, 1024, 16, 32, 16
THETA = 10000.0
EPS = 1.1920929e-07
NCORE = 8
TL = T // NCORE          # 1024 rows per core
C = 64                   # chunk length
NCH = TL // C            # 16 chunks per core
DT = D // 128            # 8 d-tiles
TT = TL // 128           # 8 t-tiles
NW = 128                 # fused projection width (padded, 32-aligned slices)


def r32(ap):
    return ap.bitcast(F32R)


# ---------------------------------------------------------------- L1 builder
NP = NCH // 2            # 8 chunk-pairs; each pair handled as 128x128 blockdiag


# consts layout: early block (ident/wall/perms - gates the front phase),
# then late block (masks/trig - needed only after projections)
O_ID, O_WALL = 0, 128            # wall: bf16 packed in 512 f32 cols
O_I16 = O_WALL + 512             # bf16 ident in 64 f32 cols
O_WA2, O_PQK, O_PBD = O_I16 + 64, O_I16 + 96, O_I16 + 160
O_M01 = O_PBD + 128              # (64,128) 0/1 pair-blockdiag mask
CW_A = O_M01 + 128               # early block width
O_MSK, O_NMSK = CW_A, CW_A + 128
O_TBD, O_TRG = CW_A + 256, CW_A + 256 + TL
CW = CW_A + 256 + 2 * TL


def build_l1():
    nc = bacc.Bacc(None, target_bir_lowering=False)
    x = nc.dram_tensor("x", (TL, D), BF16, kind="ExternalInput")
    consts = nc.dram_tensor("consts", (128, CW), F32, kind="ExternalInput")

    xT = nc.dram_tensor("xT", (D, TL), BF16, kind="ExternalOutput")
    # packed: rows 0:64 qeff (pair-blockdiag) | 64:80 obase | 80:96 gsig
    oqg = nc.dram_tensor("oqg", (128, TL), F32, kind="ExternalOutput")
    # packed: cols 0:512 amat(r0:32) | 512:640 bmat | 640:656 pca(r0:32)
    scano = nc.dram_tensor("scano", (64, 512 + NP * DV + NCH), F32,
                           kind="ExternalOutput")

    with tile.TileContext(nc) as tc:
        with (
            tc.tile_pool(name="big", bufs=1) as big,
            tc.tile_pool(name="ck", bufs=4) as ck,
            tc.tile_pool(name="ps", bufs=8, space="PSUM") as ps,
        ):
            cS = big.tile([128, CW], F32)
            nc.sync.dma_start(out=cS[:, 0:CW_A], in_=consts[:, 0:CW_A])
            nc.sync.dma_start(out=cS[:, CW_A:], in_=consts[:, CW_A:])
            ident = cS[:, O_ID:O_ID + 128]
            mskS = cS[:, O_MSK:O_MSK + 128]
            nmskS = cS[:, O_NMSK:O_NMSK + 128]
            trigbdS = cS[:, O_TBD:O_TBD + TL]
            trigS = cS[0:64, O_TRG:O_TRG + TL]
            wa2S = cS[0:DKP, O_WA2:O_WA2 + DKP]
            permS = cS[0:DKP, O_PQK:O_PQK + 2 * DKP]
            permbdS = cS[0:DKP, O_PBD:O_PBD + 128]
            ident16 = cS[:, O_I16:O_I16 + 64].bitcast(BF16)
            xbig = big.tile([128, TT, D], BF16)
            for hh in range(4):
                nc.sync.dma_start(
                    out=xbig[:, 2 * hh:2 * hh + 2, :],
                    in_=x.rearrange("(i p) d -> p i d", p=128)[:, 2 * hh:2 * hh + 2, :])

            # ---- x^T via PE transposes (SBUF -> PSUM -> SBUF, bf16) ----
            xTb = big.tile([128, DT, TL], BF16)
            for i in range(TT):
                for j in range(DT):
                    ptf = ps.tile([128, 64], F32, tag="ps")
                    pt = ptf.bitcast(BF16)
                    nc.tensor.transpose(
                        pt, xbig[:, i, 128 * j:128 * j + 128], ident16)
                    dst = xTb[:, j, 128 * i:128 * i + 128]
                    if (i + j) % 3 == 2:
                        nc.scalar.copy(dst, pt)
                    else:
                        nc.vector.tensor_copy(dst, pt)
            nc.sync.dma_start(
                out=xT.rearrange("(j p) t -> p j t", p=128), in_=xTb)
            xTs = [xTb[:, j, :] for j in range(DT)]

            # ---- fused projections: psum (113, 512) x2 ----
            # wall cols: 0:16 Wq | 16:32 Wk | 32:64 Wa1 | 64:80 Wv |
            #            96:97 Wbeta | 97:113 Wgate  (rest zero-padded)
            # ACT ops grouped by table set: {silu} {sigmoid} {ln,exp}
            vT = big.tile([DV, TL], F32)
            a1s = big.tile([DKP, TL], F32)
            gbsg = big.tile([17, TL], F32)    # [sigmoid(beta); sigmoid(gate)]
            qa = big.tile([2 * DKP, TL], F32)  # [sigmoid(-qk); sigmoid(a2)]
            la = big.tile([2 * DKP, TL], F32)  # ln(qa) = [-softplus(qk); ln a]
            pps = []
            for n in range(2):
                sl = slice(512 * n, 512 * n + 512)
                p = ps.tile([NW, 512], F32, tag="ps", name=f"pj{n}")
                for j in range(DT):
                    wj = cS[:, O_WALL + 64 * j:O_WALL + 64 * j + 64].bitcast(BF16)
                    nc.tensor.matmul(
                        p, wj, xTs[j][:, sl], start=(j == 0), stop=(j == DT - 1))
                pps.append(p)
            for n in range(2):
                sl = slice(512 * n, 512 * n + 512)
                # silu(z) = z * sigmoid(z): keeps ACT within the sigmoid table
                nc.scalar.activation(a1s[:, sl], pps[n][32:64, :], AF.Sigmoid)
                nc.vector.tensor_tensor(
                    a1s[:, sl], a1s[:, sl], pps[n][32:64, :], OP.mult)
                nc.scalar.activation(vT[:, sl], pps[n][64:80, :], AF.Sigmoid)
                nc.vector.tensor_tensor(
                    vT[:, sl], vT[:, sl], pps[n][64:80, :], OP.mult)
            pas = []
            for n in range(2):
                sl = slice(512 * n, 512 * n + 512)
                pa = ps.tile([DKP, 512], F32, tag="ps", name=f"pa{n}")
                nc.tensor.matmul(pa, wa2S, a1s[:, sl], start=True, stop=True)
                pas.append(pa)
            for n in range(2):
                sl = slice(512 * n, 512 * n + 512)
                # softplus(w) = -ln(sigmoid(-w)); sign folded into trig tables
                nc.scalar.activation(qa[0:DKP, sl], pps[n][0:32, :], AF.Sigmoid,
                                     scale=-1.0)
                nc.scalar.activation(gbsg[:, sl], pps[n][96:113, :], AF.Sigmoid)
                nc.scalar.activation(qa[DKP:, sl], pas[n], AF.Sigmoid)
            # Ln after all sigmoids (table-set grouping); split for pipelining
            nc.scalar.activation(la[:, 0:512], qa[:, 0:512], AF.Ln)
            nc.scalar.activation(la[:, 512:], qa[:, 512:], AF.Ln)
            qkmu = la[0:DKP, :]
            spT = la[DKP:, :]
            betaT = gbsg[0:1, :]

            # ---- G = within-chunk cumsum of log(alpha), duplicated to 128p
            # (4 copies so every consumer finds G at a matching base partition)
            GN4 = big.tile([128, TL], F32)
            for c in range(NCH):
                cs = slice(C * c, C * c + C)
                nc.vector.tensor_tensor_scan(
                    GN4[0:DKP, cs], spT[:, cs], spT[:, cs], 0.0, OP.add, OP.bypass)
            nc.gpsimd.tensor_copy(GN4[DKP:2 * DKP, :], GN4[0:DKP, :])
            nc.gpsimd.tensor_copy(GN4[2 * DKP:, :], GN4[0:2 * DKP, :])

            # ---- exp factors on duplicated layout (split for pipelining) ----
            eG4 = big.tile([128, TL], F32)
            eGn4 = big.tile([128, TL], F32)
            for n in range(2):
                sl = slice(512 * n, 512 * n + 512)
                nc.scalar.activation(eG4[:, sl], GN4[:, sl], AF.Exp)
                nc.scalar.activation(eGn4[2 * DKP:, sl], GN4[2 * DKP:, sl],
                                     AF.Exp, scale=-1.0)
            pCall2 = big.tile([2 * DKP, NCH], F32)
            nc.scalar.activation(pCall2, GN4[0:2 * DKP, C - 1::C], AF.Exp)

            # ---- PoPE: plain k (32p) + blockdiag-masked (128p) variants ----
            k2p = big.tile([DKP, TL], F32)         # plain k2 (for X build)
            QKbd = big.tile([128, TL], F32)        # [q ev; q od; k ev; k od]
            for n in range(2):
                sl = slice(512 * n, 512 * n + 512)
                pq = ps.tile([DKP, 512], F32, tag="ps")
                nc.tensor.matmul(pq, permS[:, DKP:], qkmu[:, sl],
                                 start=True, stop=True)
                nc.vector.tensor_tensor(k2p[:, sl], pq, trigS[DKP:, sl], OP.mult)
                pqb = ps.tile([128, 512], F32, tag="ps")
                nc.tensor.matmul(pqb, permbdS, qkmu[:, sl], start=True, stop=True)
                nc.vector.tensor_tensor(QKbd[:, sl], pqb, trigbdS[:, sl], OP.mult)
            Qtbd = big.tile([2 * DKP, TL], F32)
            Ketabd = big.tile([2 * DKP, TL], F32)
            Kkapbd = big.tile([2 * DKP, TL], F32)
            for n in range(2):
                sl = slice(512 * n, 512 * n + 512)
                nc.vector.tensor_tensor(Qtbd[:, sl], QKbd[0:2 * DKP, sl],
                                        eG4[0:2 * DKP, sl], OP.mult)
                nc.vector.tensor_tensor(Ketabd[:, sl], QKbd[2 * DKP:, sl],
                                        eGn4[2 * DKP:, sl], OP.mult)
                nc.vector.tensor_tensor(Kkapbd[:, sl], QKbd[2 * DKP:, sl],
                                        eG4[2 * DKP:, sl], OP.mult)
            Kkapp = big.tile([DKP, TL], F32)
            nc.gpsimd.tensor_tensor(Kkapp, k2p, eG4[0:DKP, :], OP.mult)
            # Kbar = Keta * exp(G_last) per chunk
            Kbarbd = big.tile([2 * DKP, TL], F32)
            for c in range(NCH):
                cs = slice(C * c, C * c + C)
                nc.gpsimd.tensor_scalar(
                    Kbarbd[:, cs], Ketabd[:, cs], pCall2[:, c:c + 1], None, OP.mult)

            # ---- beta columns: transpose (1,128) pieces -> ball (128, TT) ----
            ball = big.tile([128, TT], F32)
            for i in range(TT):
                pb = ps.tile([128, 1], F32, tag="ps")
                nc.tensor.transpose(
                    pb, betaT[0:1, 128 * i:128 * i + 128], ident[0:1, 0:1])
                nc.scalar.copy(ball[:, i:i + 1], pb)

            oqgS = big.tile([128, TL], F32)
            obS = oqgS[64:64 + DV, :]
            qeS = oqgS[0:64, :]
            m01S = cS[0:64, O_M01:O_M01 + 128]
            nc.sync.dma_start(out=oqgS[80:96, :], in_=gbsg[1:17, :])
            scanS = big.tile([64, 512 + NP * DV + NCH], F32)
            amS = scanS[0:DKP, 0:512]
            bmS = scanS[:, 512:512 + NP * DV]
            nc.vector.tensor_copy(scanS[0:DKP, 640:640 + NCH], pCall2[0:DKP, :])
            XW = DV + DKP                    # 48 solve cols
            NW2 = XW + 128                   # [X | npow] width

            # ---- per-pair delta-rule math (2 chunks per op, blockdiag) ----
            # stage-major emission: every stage streams all 8 pairs so each
            # engine sees 8 independent work items back-to-back.
            P_ = list(range(NP))
            pcs_ = [slice(128 * pi, 128 * pi + 128) for pi in P_]
            bc_ = [ball[:, pi:pi + 1] for pi in P_]

            XNs = [None] * NP
            attnTs = [None] * NP
            kbars = [None] * NP
            Xfs = [None] * NP

            for pi in P_:  # S0a: attention blocks
                pat = ps.tile([128, 128], F32, tag="ps")
                nc.tensor.matmul(pat, Ketabd[:, pcs_[pi]], Qtbd[:, pcs_[pi]],
                                 start=True, stop=True)
                attnTs[pi] = ck.tile([128, 128], F32, tag="attnT", bufs=9, name=f"attnT{pi}")
                nc.vector.tensor_tensor(attnTs[pi], pat, mskS, OP.mult)
            for pi in P_:  # S0b: N init (beta-scaled, strict-masked)
                pm = ps.tile([128, 128], F32, tag="ps")
                nc.tensor.matmul(pm, Kkapbd[:, pcs_[pi]], Ketabd[:, pcs_[pi]],
                                 start=True, stop=True)
                XNs[pi] = ck.tile([128, NW2], F32, tag="XN", bufs=18, name=f"XN{pi}")
                nc.vector.scalar_tensor_tensor(
                    XNs[pi][:, XW:], pm, bc_[pi], nmskS, OP.mult, OP.mult)
            for pi in P_:  # S0c: R = [b*V | b*Kkap], kbar_t
                pv = ps.tile([128, DV], F32, tag="ps")
                nc.tensor.transpose(pv, vT[:, pcs_[pi]], ident[0:DV, 0:DV])
                nc.vector.tensor_scalar(XNs[pi][:, 0:DV], pv, bc_[pi], None, OP.mult)
                pk = ps.tile([128, DKP], F32, tag="ps")
                nc.tensor.transpose(pk, Kkapp[:, pcs_[pi]], ident[0:DKP, 0:DKP])
                nc.vector.tensor_scalar(XNs[pi][:, DV:XW], pk, bc_[pi], None, OP.mult)
                pkb = ps.tile([128, 2 * DKP], F32, tag="ps")
                nc.tensor.transpose(pkb, Kbarbd[:, pcs_[pi]],
                                    ident[0:2 * DKP, 0:2 * DKP])
                kbars[pi] = ck.tile([128, 2 * DKP], F32, tag="kbar_t", bufs=9, name=f"kb{pi}")
                nc.scalar.copy(kbars[pi], pkb)

            # X = (I+M)^-1 R via product of (I + N^(2^j)); npow rides in XN
            for j in range(6):
                npTs = [None] * NP
                for pi in P_:
                    ptp = ps.tile([128, 128], F32, tag="ps")
                    nc.tensor.transpose(ptp, XNs[pi][:, XW:], ident)
                    npTs[pi] = ck.tile([128, 128], F32, tag="npT", bufs=10, name=f"npT{pi}")
                    if pi % 2 == 0:
                        nc.scalar.copy(npTs[pi], ptp)
                    else:
                        nc.vector.tensor_copy(npTs[pi], ptp)
                if j < 5:
                    for pi in P_:
                        px = ps.tile([128, NW2], F32, tag="ps")
                        nc.tensor.matmul(px, npTs[pi], XNs[pi],
                                         start=True, stop=True)
                        XN2 = ck.tile([128, NW2], F32, tag="XN", bufs=18)
                        nc.vector.tensor_tensor(
                            XN2[:, 0:XW], XNs[pi][:, 0:XW], px[:, 0:XW], OP.add)
                        if (j + pi) % 2 == 0:
                            nc.scalar.copy(XN2[:, XW:], px[:, XW:])
                        else:
                            nc.vector.tensor_copy(XN2[:, XW:], px[:, XW:])
                        XNs[pi] = XN2
                else:
                    for pi in P_:
                        px = ps.tile([128, XW], F32, tag="ps")
                        nc.tensor.matmul(px, npTs[pi], XNs[pi][:, 0:XW],
                                         start=True, stop=True)
                        Xfs[pi] = ck.tile([128, XW], F32, tag="Xf", bufs=9, name=f"Xf{pi}")
                        nc.vector.tensor_tensor(
                            Xfs[pi], XNs[pi][:, 0:XW], px, OP.add)

            for pi in P_:  # S7: chunk-local outputs + state maps
                pob = ps.tile([DV, 128], F32, tag="ps")
                nc.tensor.matmul(pob, Xfs[pi][:, 0:DV], attnTs[pi],
                                 start=True, stop=True)
                nc.scalar.copy(obS[:, pcs_[pi]], pob)
                # qeff in pair-blockdiag form: rows 0:32 even / 32:64 odd chunk
                pqb = ps.tile([64, 128], F32, tag="ps")
                nc.tensor.matmul(pqb[0:DKP, :], Xfs[pi][:, DV:], attnTs[pi],
                                 start=True, stop=True)
                nc.tensor.matmul(pqb[DKP:, :], Xfs[pi][:, DV:], attnTs[pi],
                                 start=True, stop=True, skip_group_check=True)
                qtmp = ck.tile([64, 128], F32, tag="qtmp", bufs=4,
                               name=f"qtmp{pi}")
                nc.vector.tensor_tensor(qtmp, pqb, m01S, OP.mult)
                nc.gpsimd.tensor_tensor(
                    qeS[:, pcs_[pi]], Qtbd[:, pcs_[pi]], qtmp, OP.subtract)
                pa2 = ps.tile([DKP, 2 * DKP], F32, tag="ps")
                nc.tensor.matmul(pa2, Xfs[pi][:, DV:], kbars[pi],
                                 start=True, stop=True)
                nc.vector.tensor_copy(
                    amS[:, 2 * DKP * pi:2 * DKP * pi + 2 * DKP], pa2)
                pbm = ps.tile([2 * DKP, DV], F32, tag="ps")
                nc.tensor.matmul(pbm, kbars[pi], Xfs[pi][:, 0:DV],
                                 start=True, stop=True)
                nc.scalar.copy(bmS[:, DV * pi:DV * pi + DV], pbm)

            nc.sync.dma_start(out=oqg[:, :], in_=oqgS)
            nc.sync.dma_start(out=scano[:, :], in_=scanS)
    nc.compile()
    return nc


# ---------------------------------------------------------------- L2 builder
Q_SE = TL
Q_PNW = Q_SE + NCH * DV
Q_WOUT = Q_PNW + 1
Q_ONES = Q_WOUT + D
C2W = Q_ONES + 128


def build_l2():
    nc = bacc.Bacc(None, target_bir_lowering=False)
    x = nc.dram_tensor("x", (TL, D), F32, kind="ExternalInput")
    xT = nc.dram_tensor("xT", (D, TL), BF16, kind="ExternalInput")
    # packed consts2: cols 0:TL rows[0:16 obase |16:32 gsig |32:64 qeff],
    # then sentry (rows 32:64), pnw, wout, ones
    consts2 = nc.dram_tensor("consts2", (128, C2W), F32, kind="ExternalInput")
    wout = nc.dram_tensor("wout", (DV, D), F32R, kind="ExternalInput")
    wg = nc.dram_tensor("wg", (D, D), F8, kind="ExternalInput")
    wu = nc.dram_tensor("wu", (D, D), F8, kind="ExternalInput")
    wd = nc.dram_tensor("wd", (D, D), F8, kind="ExternalInput")
    y = nc.dram_tensor("y", (TL, D), F32, kind="ExternalOutput")

    with tile.TileContext(nc) as tc:
        with (
            tc.tile_pool(name="big", bufs=1) as big,
            tc.tile_pool(name="work", bufs=3) as work,
            tc.tile_pool(name="psr", bufs=2, space="PSUM") as psr,
            tc.tile_pool(name="psb", bufs=2, space="PSUM") as psb,
            tc.tile_pool(name="psm", bufs=4, space="PSUM") as psm,
        ):
            cS2 = big.tile([128, C2W], F32)
            nc.sync.dma_start(out=cS2, in_=consts2[:, :])
            onesS = cS2[:, Q_ONES:Q_ONES + 128]
            obS = cS2[64:64 + DV, 0:TL]
            qeS = cS2[0:64, 0:TL]
            gsS = big.tile([DV, TL], F32)
            nc.sync.dma_start(out=gsS, in_=cS2[80:96, 0:TL])
            seS = cS2[0:64, Q_SE:Q_SE + NP * DV]
            woutS = big.tile([DV, D], F32R)
            nc.sync.dma_start(out=woutS, in_=wout[:, :])
            pnwS = cS2[0:DV, Q_PNW:Q_PNW + 1]
            epsS = big.tile([1, 1], F32)
            nc.vector.memset(epsS, EPS)
            x1b = big.tile([128, DT, TL], BF16)
            for hh in range(2):
                nc.sync.dma_start(
                    out=x1b[:, 4 * hh:4 * hh + 4, :],
                    in_=xT.rearrange("(j p) t -> p j t", p=128)
                    [:, 4 * hh:4 * hh + 4, :])
            x1s = [x1b[:, j, :] for j in range(DT)]
            # full-weight + x preloads (no deps -> overlap with o/norm phase)
            wgS = big.tile([128, DT, D], F8)
            nc.sync.dma_start(out=wgS, in_=wg.rearrange("(j p) f -> p j f", p=128))
            wuS = big.tile([128, DT, D], F8)
            nc.sync.dma_start(out=wuS, in_=wu.rearrange("(j p) f -> p j f", p=128))
            wds = big.tile([128, DT, D], F8)
            nc.sync.dma_start(out=wds, in_=wd.rearrange("(f p) d -> p f d", p=128))
            xbig = big.tile([128, TT, D], F32)
            nc.sync.dma_start(out=xbig, in_=x.rearrange("(i p) d -> p i d", p=128))

            # ---- o^T assembly (blockdiag sentry, one matmul per pair) ----
            oT = big.tile([DV, TL], F32)
            for pi in range(NP):
                pcs = slice(128 * pi, 128 * pi + 128)
                po = psr.tile([DV, 128], F32, tag="red")
                nc.tensor.matmul(po, seS[:, DV * pi:DV * pi + DV], qeS[:, pcs],
                                 start=True, stop=True)
                nc.vector.tensor_tensor(oT[:, pcs], obS[:, pcs], po, OP.add)

            # ---- post rmsnorm + gate:  og = rms(o)*pnw*gsig ----
            osq = big.tile([DV, TL], F32)
            nc.scalar.activation(osq, oT, AF.Square)
            og = big.tile([DV, TL], F32R)
            for n in range(2):
                sl = slice(512 * n, 512 * n + 512)
                prs = psr.tile([1, 512], F32, tag="red")
                nc.tensor.matmul(prs, onesS[0:DV, 0:1], osq[:, sl],
                                 start=True, stop=True)
                rq = work.tile([1, 512], F32, tag="rq")
                nc.scalar.activation(rq, prs, AF.Sqrt, scale=1.0 / DV, bias=epsS[:, :])
                rr = work.tile([1, 512], F32, tag="rr")
                nc.vector.reciprocal(rr, rq)
                pbv = psr.tile([DV, 512], F32, tag="red")
                nc.tensor.matmul(pbv, onesS[0:1, 0:DV], rr,
                                 start=True, stop=True)
                t1 = work.tile([DV, 512], F32, tag="t1")
                nc.vector.tensor_tensor(t1, oT[:, sl], pbv, OP.mult)
                t2 = work.tile([DV, 512], F32, tag="t2")
                nc.vector.tensor_scalar(t2, t1, pnwS[:, :], None, OP.mult)
                nc.vector.tensor_tensor(og[:, sl], t2, gsS[:, sl], OP.mult)

            # ---- x1^T = x^T + Wout^T og  (in-place into x1s) ----
            for j in range(DT):
                for n in range(2):
                    sl = slice(512 * n, 512 * n + 512)
                    px1 = psm.tile([128, 512], F32, tag="mm")
                    nc.tensor.matmul(px1, woutS[:, 128 * j:128 * j + 128],
                                     og[:, sl], start=True, stop=True)
                    nc.vector.tensor_tensor(x1s[j][:, sl], x1s[j][:, sl], px1, OP.add)

            # ---- ffn rmsnorm -> h (in-place into x1s) ----
            rb = []
            for n in range(2):
                sl = slice(512 * n, 512 * n + 512)
                ph = psr.tile([1, 512], F32, tag="red")
                for j in range(DT):
                    sq = work.tile([128, 512], F32, tag="sq")
                    nc.scalar.activation(sq, x1s[j][:, sl], AF.Square)
                    nc.tensor.matmul(ph, onesS[:, 0:1], sq,
                                     start=(j == 0), stop=(j == DT - 1))
                r1q = work.tile([1, 512], F32, tag="r1q")
                nc.scalar.activation(r1q, ph, AF.Sqrt, scale=1.0 / D, bias=epsS[:, :])
                r1 = work.tile([1, 512], F32, tag="r1")
                nc.vector.reciprocal(r1, r1q)
                pbb = psb.tile([128, 512], F32, tag="bcb")
                nc.tensor.matmul(pbb, onesS[0:1, :], r1,
                                 start=True, stop=True)
                rbn = big.tile([128, 512], F32, name=f"rb{n}")
                nc.scalar.activation(rbn, pbb, AF.Copy)
                rb.append(rbn)
            h8 = big.tile([128, DT, TL], F8)
            for j in range(DT):
                for n in range(2):
                    sl = slice(512 * n, 512 * n + 512)
                    eng = nc.gpsimd if (2 * j + n) % 3 == 2 else nc.vector
                    eng.tensor_tensor(h8[:, j, sl], x1s[j][:, sl], rb[n], OP.mult)

            # ---- gate/up -> z (fp8 DoubleRow matmuls); n-outer so the down
            # projection for token-slice n can overlap gate/up of slice n+1
            zs8 = big.tile([128, DT, TL], F8)
            for n in range(2):
                sl = slice(512 * n, 512 * n + 512)
                for f in range(DT):
                    fs = slice(128 * f, 128 * f + 128)
                    pg = psm.tile([128, 512], F32, tag="mm")
                    for j in range(DT // 2):
                        nc.tensor.matmul(pg, wgS[:, 2 * j:2 * j + 2, fs],
                                         h8[:, 2 * j:2 * j + 2, sl],
                                         start=(j == 0), stop=(j == DT // 2 - 1),
                                         perf_mode=DR)
                    pu = psm.tile([128, 512], F32, tag="mm")
                    for j in range(DT // 2):
                        nc.tensor.matmul(pu, wuS[:, 2 * j:2 * j + 2, fs],
                                         h8[:, 2 * j:2 * j + 2, sl],
                                         start=(j == 0), stop=(j == DT // 2 - 1),
                                         perf_mode=DR)
                    gs = work.tile([128, 512], F32, tag="gs")
                    nc.scalar.activation(gs, pg, AF.Silu)
                    nc.vector.tensor_tensor(zs8[:, f, sl], pu, gs, OP.mult)

                # ---- down + wout-term + x residual -> y (tiles of slice n) ----
                for i in range(4 * n, 4 * n + 4):
                    ts = slice(128 * i, 128 * i + 128)
                    for n2 in range(2):
                        sl2 = slice(512 * n2, 512 * n2 + 512)
                        pd = psm.tile([128, 512], F32, tag="mm")
                        for f in range(DT // 2):
                            nc.tensor.matmul(pd, zs8[:, 2 * f:2 * f + 2, ts],
                                             wds[:, 2 * f:2 * f + 2, sl2],
                                             start=(f == 0), stop=False,
                                             perf_mode=DR)
                        nc.tensor.matmul(pd, og[:, ts], woutS[:, sl2],
                                         start=False, stop=True,
                                         skip_group_check=True)
                        yo = work.tile([128, 512], F32, tag="yo")
                        nc.vector.tensor_tensor(yo, pd, xbig[:, i, sl2], OP.add)
                        nc.sync.dma_start(out=y[ts, sl2], in_=yo)
    nc.compile()
    return nc


# ---------------------------------------------------------------- host glue
_CACHE = {}
LAST_RESULTS = {}  # populated with BassKernelResults per launch (for test.py)


def kernel(**inputs):
    x_seq = np.ascontiguousarray(np.asarray(inputs["x_seq"], np.float32))

    def sigmoid(z):
        return 1.0 / (1.0 + np.exp(-z))

    positions = np.arange(T, dtype=np.float32)
    freqs = THETA ** (np.arange(DK, dtype=np.float32) / DK)
    phi = positions[:, None] * freqs[None, :]
    psi = 2.0 * math.pi * sigmoid(np.asarray(inputs["pope_delta_raw"], np.float32))
    trig_full = -np.concatenate(
        [np.cos(phi).T, np.sin(phi).T, np.cos(phi - psi).T, np.sin(phi - psi).T],
        axis=0).astype(np.float32)  # (64, T); negated: device mu = -softplus
    # blockdiag-masked trig: [q*even; q*odd; k*even; k*odd] per chunk parity
    par = ((positions.astype(np.int64) // C) % 2).astype(np.float32)  # (T,)
    evm, odm = (1.0 - par)[None, :], par[None, :]
    trig_bd = np.concatenate(
        [trig_full[0:DKP] * evm, trig_full[0:DKP] * odm,
         trig_full[DKP:] * evm, trig_full[DKP:] * odm], axis=0).astype(np.float32)
    wall = np.zeros((D, NW), np.float32)
    wall[:, 0:16] = np.asarray(inputs["Wq"], np.float32)
    wall[:, 16:32] = np.asarray(inputs["Wk"], np.float32)
    wall[:, 32:64] = np.asarray(inputs["Wa1"], np.float32)
    wall[:, 64:80] = np.asarray(inputs["Wv"], np.float32)
    wall[:, 96:97] = np.asarray(inputs["Wbeta"], np.float32)
    wall[:, 97:113] = np.asarray(inputs["Wgate"], np.float32)
    i128 = np.eye(128, dtype=np.float32)
    permqk = np.zeros((DKP, 2 * DKP), np.float32)
    for r in range(DKP):
        permqk[r % DK, r] = 1.0          # qrep: out row r <- qmu row r%16
        permqk[DK + r % DK, DKP + r] = 1.0  # krep
    permbd = np.zeros((DKP, 128), np.float32)
    for cc in range(128):
        r = cc % DKP
        permbd[(r % DK) if cc < 64 else (DK + r % DK), cc] = 1.0

    mskP = np.zeros((128, 128), np.float32)
    nmskP = np.zeros((128, 128), np.float32)
    for b in range(2):
        bs = slice(C * b, C * b + C)
        mskP[bs, bs] = np.triu(np.ones((C, C), np.float32), 0)
        nmskP[bs, bs] = np.tril(-np.ones((C, C), np.float32), -1)

    if "l1" not in _CACHE:
        _CACHE["l1"] = build_l1()
    bf16 = mybir.dt.np(BF16)
    x16 = x_seq.astype(bf16)
    # bf16 data reinterpreted as f32 words for the packed consts tensor
    wallw = np.ascontiguousarray(
        wall.astype(bf16).reshape(DT, 128, NW).transpose(1, 0, 2)
        .reshape(128, DT * NW)).view(np.float32)
    i16w = np.ascontiguousarray(i128.astype(bf16)).view(np.float32)
    in1 = []
    for m in range(NCORE):
        sl = slice(TL * m, TL * m + TL)
        cs = np.zeros((128, CW), np.float32)
        cs[:, O_ID:O_ID + 128] = i128
        cs[:, O_MSK:O_MSK + 128] = mskP
        cs[:, O_NMSK:O_NMSK + 128] = nmskP
        cs[:, O_TBD:O_TBD + TL] = trig_bd[:, sl]
        cs[0:64, O_TRG:O_TRG + TL] = trig_full[:, sl]
        cs[:, O_WALL:O_WALL + 512] = wallw.view(np.float32)
        cs[:, O_I16:O_I16 + 64] = i16w.view(np.float32)
        cs[0:DKP, O_WA2:O_WA2 + DKP] = np.asarray(inputs["Wa2"], np.float32)
        cs[0:DKP, O_PQK:O_PQK + 2 * DKP] = permqk
        cs[0:DKP, O_PBD:O_PBD + 128] = permbd
        cs[0:DKP, O_M01:O_M01 + 64] = 1.0
        cs[DKP:64, O_M01 + 64:O_M01 + 128] = 1.0
        in1.append({"x": np.ascontiguousarray(x16[sl]), "consts": cs})
    r1 = run_bass_kernel_spmd(_CACHE["l1"], in1, core_ids=list(range(NCORE)))
    LAST_RESULTS["l1"] = r1
    res1 = r1.results

    # host chunk-state scan (128 tiny steps); sentry in pair-blockdiag rows
    S = np.zeros((DKP, DV), np.float32)
    sentries = []
    for m in range(NCORE):
        se = np.zeros((64, NP * DV), np.float32)
        sc = res1[m]["scano"]
        am, bm = sc[0:DKP, 0:512], sc[:, 512:512 + NP * DV]
        pc = sc[0:DKP, 640:640 + NCH]
        for c in range(NCH):
            p, odd = c // 2, c % 2
            se[DKP * odd:DKP * odd + DKP, DV * p:DV * p + DV] = S
            wtk = am[:, 2 * DKP * p + DKP * odd:2 * DKP * p + DKP * odd + DKP]
            AT = np.diag(pc[:, c]) - wtk
            B = bm[DKP * odd:DKP * odd + DKP, DV * p:DV * p + DV]
            S = AT.T @ S + B
        sentries.append(se)

    f8np = mybir.dt.np(F8)
    ffnw = np.asarray(inputs["ffn_norm_w"], np.float32)[:, None]
    wgm = np.ascontiguousarray(
        (ffnw * np.asarray(inputs["Wffn_gate"], np.float32)).astype(f8np))
    wum = np.ascontiguousarray(
        (ffnw * np.asarray(inputs["Wffn_up"], np.float32)).astype(f8np))
    wdm = np.ascontiguousarray(
        np.asarray(inputs["Wffn_down"], np.float32).astype(f8np))
    onesc = np.ones((128, 128), np.float32)

    if "l2" not in _CACHE:
        _CACHE["l2"] = build_l2()
    in2 = []
    for m in range(NCORE):
        sl = slice(TL * m, TL * m + TL)
        c2 = np.zeros((128, C2W), np.float32)
        c2[:, 0:TL] = res1[m]["oqg"]
        c2[0:64, Q_SE:Q_SE + NP * DV] = sentries[m]
        c2[0:DV, Q_PNW] = np.asarray(inputs["post_norm_w"], np.float32)
        c2[:, Q_ONES:Q_ONES + 128] = onesc
        in2.append({
            "x": np.ascontiguousarray(x_seq[sl]),
            "xT": res1[m]["xT"], "consts2": c2,
            "wout": np.ascontiguousarray(inputs["Wout"]),
            "wg": wgm, "wu": wum, "wd": wdm,
        })
    r2 = run_bass_kernel_spmd(_CACHE["l2"], in2, core_ids=list(range(NCORE)))
    LAST_RESULTS["l2"] = r2
    res2 = r2.results
    return np.concatenate([res2[m]["y"] for m in range(NCORE)], axis=0)



# revision 90
# speedup vs baseline: 1.0175x; 1.0175x over previous
"""Trainium2 Bass kernel for MiniKDALayer (chunked delta-rule + gated FFN).

Sequence-parallel over 8 cores (T=8192 -> 1024 rows/core):
  L1 kernel: x^T transpose, fused projections, PoPE, per-chunk (C=64)
     delta-rule quantities via the WY chunked form -> per-chunk affine state
     maps (A^T, B) and chunk-local output pieces (Obase^T, Qeff^T).
  Host: 128-step chunk-state scan over tiny (32,16) states.
  L2 kernel: o assembly, post-RMSNorm, gating, Wout+residual, FFN with
     fp32r matmuls, final residual, straight (t-major) output.
"""
import math

import numpy as np

import concourse.bass as bass
import concourse.bacc as bacc
import concourse.mybir as mybir
import concourse.tile as tile
from concourse.bass_utils import run_bass_kernel_spmd

F32 = mybir.dt.float32
F32R = mybir.dt.float32r
BF16 = mybir.dt.bfloat16
F8 = mybir.dt.float8e4
DR = mybir.MatmulPerfMode.DoubleRow
AF = mybir.ActivationFunctionType
OP = mybir.AluOpType

T, D, DK, DKP, DV = 8192, 1024, 16, 32, 16
THETA = 10000.0
EPS = 1.1920929e-07
NCORE = 8
TL = T // NCORE          # 1024 rows per core
C = 64                   # chunk length
NCH = TL // C            # 16 chunks per core
DT = D // 128            # 8 d-tiles
TT = TL // 128           # 8 t-tiles
NW = 128                 # fused projection width (padded, 32-aligned slices)


def r32(ap):
    return ap.bitcast(F32R)


# ---------------------------------------------------------------- L1 builder
NP = NCH // 2            # 8 chunk-pairs; each pair handled as 128x128 blockdiag


# consts layout: early block (ident/wall/perms - gates the front phase),
# then late block (masks/trig - needed only after projections)
O_ID, O_WALL = 0, 128            # wall: bf16 packed in 512 f32 cols
O_I16 = O_WALL + 512             # bf16 ident in 64 f32 cols
O_WA2, O_PQK, O_PBD = O_I16 + 64, O_I16 + 96, O_I16 + 160
O_M01 = O_PBD + 128              # (64,128) 0/1 pair-blockdiag mask
CW_A = O_M01 + 128               # early block width
O_MSK, O_NMSK = CW_A, CW_A + 128
O_TBD, O_TRG = CW_A + 256, CW_A + 256 + TL
CW = CW_A + 256 + 2 * TL


def build_l1():
    nc = bacc.Bacc(None, target_bir_lowering=False)
    x = nc.dram_tensor("x", (TL, D), BF16, kind="ExternalInput")
    consts = nc.dram_tensor("consts", (128, CW), F32, kind="ExternalInput")

    xT = nc.dram_tensor("xT", (D, TL), BF16, kind="ExternalOutput")
    # packed: rows 0:64 qeff (pair-blockdiag) | 64:80 obase | 80:96 gsig
    oqg = nc.dram_tensor("oqg", (128, TL), F32, kind="ExternalOutput")
    # packed: cols 0:512 amat(r0:32) | 512:640 bmat | 640:656 pca(r0:32)
    scano = nc.dram_tensor("scano", (64, 512 + NP * DV + NCH), F32,
                           kind="ExternalOutput")

    with tile.TileContext(nc) as tc:
        with (
            tc.tile_pool(name="big", bufs=1) as big,
            tc.tile_pool(name="ck", bufs=4) as ck,
            tc.tile_pool(name="ps", bufs=8, space="PSUM") as ps,
        ):
            cS = big.tile([128, CW], F32)
            nc.sync.dma_start(out=cS[:, 0:CW_A], in_=consts[:, 0:CW_A])
            nc.sync.dma_start(out=cS[:, CW_A:], in_=consts[:, CW_A:])
            ident = cS[:, O_ID:O_ID + 128]
            mskS = cS[:, O_MSK:O_MSK + 128]
            nmskS = cS[:, O_NMSK:O_NMSK + 128]
            trigbdS = cS[:, O_TBD:O_TBD + TL]
            trigS = cS[0:64, O_TRG:O_TRG + TL]
            wa2S = cS[0:DKP, O_WA2:O_WA2 + DKP]
            permS = cS[0:DKP, O_PQK:O_PQK + 2 * DKP]
            permbdS = cS[0:DKP, O_PBD:O_PBD + 128]
            ident16 = cS[:, O_I16:O_I16 + 64].bitcast(BF16)
            xbig = big.tile([128, TT, D], BF16)
            for hh in range(4):
                nc.sync.dma_start(
                    out=xbig[:, 2 * hh:2 * hh + 2, :],
                    in_=x.rearrange("(i p) d -> p i d", p=128)[:, 2 * hh:2 * hh + 2, :])

            # ---- x^T via PE transposes (SBUF -> PSUM -> SBUF, bf16) ----
            xTb = big.tile([128, DT, TL], BF16)
            for i in range(TT):
                for j in range(DT):
                    ptf = ps.tile([128, 64], F32, tag="ps")
                    pt = ptf.bitcast(BF16)
                    nc.tensor.transpose(
                        pt, xbig[:, i, 128 * j:128 * j + 128], ident16)
                    dst = xTb[:, j, 128 * i:128 * i + 128]
                    if (i + j) % 3 == 2:
                        nc.scalar.copy(dst, pt)
                    else:
                        nc.vector.tensor_copy(dst, pt)
            nc.sync.dma_start(
                out=xT.rearrange("(j p) t -> p j t", p=128), in_=xTb)
            xTs = [xTb[:, j, :] for j in range(DT)]

            # ---- fused projections: psum (113, 512) x2 ----
            # wall cols: 0:16 Wq | 16:32 Wk | 32:64 Wa1 | 64:80 Wv |
            #            96:97 Wbeta | 97:113 Wgate  (rest zero-padded)
            # ACT ops grouped by table set: {silu} {sigmoid} {ln,exp}
            vT = big.tile([DV, TL], F32)
            a1s = big.tile([DKP, TL], F32)
            gbsg = big.tile([17, TL], F32)    # [sigmoid(beta); sigmoid(gate)]
            qa = big.tile([2 * DKP, TL], F32)  # [sigmoid(-qk); sigmoid(a2)]
            la = big.tile([2 * DKP, TL], F32)  # ln(qa) = [-softplus(qk); ln a]
            pps = []
            for n in range(2):
                sl = slice(512 * n, 512 * n + 512)
                p = ps.tile([NW, 512], F32, tag="ps", name=f"pj{n}")
                for j in range(DT):
                    wj = cS[:, O_WALL + 64 * j:O_WALL + 64 * j + 64].bitcast(BF16)
                    nc.tensor.matmul(
                        p, wj, xTs[j][:, sl], start=(j == 0), stop=(j == DT - 1))
                pps.append(p)
            for n in range(2):
                sl = slice(512 * n, 512 * n + 512)
                # silu(z) = z * sigmoid(z): keeps ACT within the sigmoid table
                nc.scalar.activation(a1s[:, sl], pps[n][32:64, :], AF.Sigmoid)
                nc.vector.tensor_tensor(
                    a1s[:, sl], a1s[:, sl], pps[n][32:64, :], OP.mult)
                nc.scalar.activation(vT[:, sl], pps[n][64:80, :], AF.Sigmoid)
                nc.vector.tensor_tensor(
                    vT[:, sl], vT[:, sl], pps[n][64:80, :], OP.mult)
            pas = []
            for n in range(2):
                sl = slice(512 * n, 512 * n + 512)
                pa = ps.tile([DKP, 512], F32, tag="ps", name=f"pa{n}")
                nc.tensor.matmul(pa, wa2S, a1s[:, sl], start=True, stop=True)
                pas.append(pa)
            for n in range(2):
                sl = slice(512 * n, 512 * n + 512)
                # softplus(w) = -ln(sigmoid(-w)); sign folded into trig tables
                nc.scalar.activation(qa[0:DKP, sl], pps[n][0:32, :], AF.Sigmoid,
                                     scale=-1.0)
                nc.scalar.activation(gbsg[:, sl], pps[n][96:113, :], AF.Sigmoid)
                nc.scalar.activation(qa[DKP:, sl], pas[n], AF.Sigmoid)
            # Ln after all sigmoids (table-set grouping); split for pipelining
            nc.scalar.activation(la[:, 0:512], qa[:, 0:512], AF.Ln)
            nc.scalar.activation(la[:, 512:], qa[:, 512:], AF.Ln)
            qkmu = la[0:DKP, :]
            spT = la[DKP:, :]
            betaT = gbsg[0:1, :]

            # ---- G = within-chunk cumsum of log(alpha), duplicated to 128p
            # (4 copies so every consumer finds G at a matching base partition)
            GN4 = big.tile([128, TL], F32)
            for c in range(NCH):
                cs = slice(C * c, C * c + C)
                nc.vector.tensor_tensor_scan(
                    GN4[0:DKP, cs], spT[:, cs], spT[:, cs], 0.0, OP.add, OP.bypass)
            nc.gpsimd.tensor_copy(GN4[DKP:2 * DKP, :], GN4[0:DKP, :])
            nc.gpsimd.tensor_copy(GN4[2 * DKP:, :], GN4[0:2 * DKP, :])

            # ---- exp factors on duplicated layout (split for pipelining) ----
            eG4 = big.tile([128, TL], F32)
            eGn4 = big.tile([128, TL], F32)
            for n in range(2):
                sl = slice(512 * n, 512 * n + 512)
                nc.scalar.activation(eG4[:, sl], GN4[:, sl], AF.Exp)
                nc.scalar.activation(eGn4[2 * DKP:, sl], GN4[2 * DKP:, sl],
                                     AF.Exp, scale=-1.0)
            pCall2 = big.tile([2 * DKP, NCH], F32)
            nc.scalar.activation(pCall2, GN4[0:2 * DKP, C - 1::C], AF.Exp)

            # ---- PoPE: plain k (32p) + blockdiag-masked (128p) variants ----
            k2p = big.tile([DKP, TL], F32)         # plain k2 (for X build)
            QKbd = big.tile([128, TL], F32)        # [q ev; q od; k ev; k od]
            for n in range(2):
                sl = slice(512 * n, 512 * n + 512)
                pq = ps.tile([DKP, 512], F32, tag="ps")
                nc.tensor.matmul(pq, permS[:, DKP:], qkmu[:, sl],
                                 start=True, stop=True)
                nc.vector.tensor_tensor(k2p[:, sl], pq, trigS[DKP:, sl], OP.mult)
                pqb = ps.tile([128, 512], F32, tag="ps")
                nc.tensor.matmul(pqb, permbdS, qkmu[:, sl], start=True, stop=True)
                nc.vector.tensor_tensor(QKbd[:, sl], pqb, trigbdS[:, sl], OP.mult)
            Qtbd = big.tile([2 * DKP, TL], F32)
            Ketabd = big.tile([2 * DKP, TL], F32)
            Kkapbd = big.tile([2 * DKP, TL], F32)
            for n in range(2):
                sl = slice(512 * n, 512 * n + 512)
                nc.vector.tensor_tensor(Qtbd[:, sl], QKbd[0:2 * DKP, sl],
                                        eG4[0:2 * DKP, sl], OP.mult)
                nc.vector.tensor_tensor(Ketabd[:, sl], QKbd[2 * DKP:, sl],
                                        eGn4[2 * DKP:, sl], OP.mult)
                nc.vector.tensor_tensor(Kkapbd[:, sl], QKbd[2 * DKP:, sl],
                                        eG4[2 * DKP:, sl], OP.mult)
            Kkapp = big.tile([DKP, TL], F32)
            nc.gpsimd.tensor_tensor(Kkapp, k2p, eG4[0:DKP, :], OP.mult)
            # Kbar = Keta * exp(G_last) per chunk
            Kbarbd = big.tile([2 * DKP, TL], F32)
            for c in range(NCH):
                cs = slice(C * c, C * c + C)
                nc.gpsimd.tensor_scalar(
                    Kbarbd[:, cs], Ketabd[:, cs], pCall2[:, c:c + 1], None, OP.mult)

            # ---- beta columns: transpose (1,128) pieces -> ball (128, TT) ----
            ball = big.tile([128, TT], F32)
            for i in range(TT):
                pb = ps.tile([128, 1], F32, tag="ps")
                nc.tensor.transpose(
                    pb, betaT[0:1, 128 * i:128 * i + 128], ident[0:1, 0:1])
                nc.scalar.copy(ball[:, i:i + 1], pb)

            oqgS = big.tile([128, TL], F32)
            obS = oqgS[64:64 + DV, :]
            qeS = oqgS[0:64, :]
            m01S = cS[0:64, O_M01:O_M01 + 128]
            nc.sync.dma_start(out=oqgS[80:96, :], in_=gbsg[1:17, :])
            scanS = big.tile([64, 512 + NP * DV + NCH], F32)
            amS = scanS[0:DKP, 0:512]
            bmS = scanS[:, 512:512 + NP * DV]
            nc.vector.tensor_copy(scanS[0:DKP, 640:640 + NCH], pCall2[0:DKP, :])
            XW = DV + DKP                    # 48 solve cols
            NW2 = XW + 128                   # [X | npow] width

            # ---- per-pair delta-rule math (2 chunks per op, blockdiag) ----
            # stage-major emission: every stage streams all 8 pairs so each
            # engine sees 8 independent work items back-to-back.
            P_ = list(range(NP))
            pcs_ = [slice(128 * pi, 128 * pi + 128) for pi in P_]
            bc_ = [ball[:, pi:pi + 1] for pi in P_]

            XNs = [None] * NP
            attnTs = [None] * NP
            kbars = [None] * NP
            Xfs = [None] * NP

            for pi in P_:  # S0a: attention blocks
                pat = ps.tile([128, 128], F32, tag="ps")
                nc.tensor.matmul(pat, Ketabd[:, pcs_[pi]], Qtbd[:, pcs_[pi]],
                                 start=True, stop=True)
                attnTs[pi] = ck.tile([128, 128], F32, tag="attnT", bufs=9, name=f"attnT{pi}")
                nc.vector.tensor_tensor(attnTs[pi], pat, mskS, OP.mult)
            for pi in P_:  # S0b: N init (beta-scaled, strict-masked)
                pm = ps.tile([128, 128], F32, tag="ps")
                nc.tensor.matmul(pm, Kkapbd[:, pcs_[pi]], Ketabd[:, pcs_[pi]],
                                 start=True, stop=True)
                XNs[pi] = ck.tile([128, NW2], F32, tag="XN", bufs=18, name=f"XN{pi}")
                nc.vector.scalar_tensor_tensor(
                    XNs[pi][:, XW:], pm, bc_[pi], nmskS, OP.mult, OP.mult)
            for pi in P_:  # S0c: R = [b*V | b*Kkap], kbar_t
                pv = ps.tile([128, DV], F32, tag="ps")
                nc.tensor.transpose(pv, vT[:, pcs_[pi]], ident[0:DV, 0:DV])
                nc.vector.tensor_scalar(XNs[pi][:, 0:DV], pv, bc_[pi], None, OP.mult)
                pk = ps.tile([128, DKP], F32, tag="ps")
                nc.tensor.transpose(pk, Kkapp[:, pcs_[pi]], ident[0:DKP, 0:DKP])
                nc.vector.tensor_scalar(XNs[pi][:, DV:XW], pk, bc_[pi], None, OP.mult)
                pkb = ps.tile([128, 2 * DKP], F32, tag="ps")
                nc.tensor.transpose(pkb, Kbarbd[:, pcs_[pi]],
                                    ident[0:2 * DKP, 0:2 * DKP])
                kbars[pi] = ck.tile([128, 2 * DKP], F32, tag="kbar_t", bufs=9, name=f"kb{pi}")
                nc.scalar.copy(kbars[pi], pkb)

            # X = (I+M)^-1 R via product of (I + N^(2^j)); npow rides in XN
            for j in range(6):
                npTs = [None] * NP
                for pi in P_:
                    ptp = ps.tile([128, 128], F32, tag="ps")
                    nc.tensor.transpose(ptp, XNs[pi][:, XW:], ident)
                    npTs[pi] = ck.tile([128, 128], F32, tag="npT", bufs=10, name=f"npT{pi}")
                    if pi % 2 == 0:
                        nc.scalar.copy(npTs[pi], ptp)
                    else:
                        nc.vector.tensor_copy(npTs[pi], ptp)
                if j < 5:
                    for pi in P_:
                        px = ps.tile([128, NW2], F32, tag="ps")
                        nc.tensor.matmul(px, npTs[pi], XNs[pi],
                                         start=True, stop=True)
                        XN2 = ck.tile([128, NW2], F32, tag="XN", bufs=18)
                        nc.vector.tensor_tensor(
                            XN2[:, 0:XW], XNs[pi][:, 0:XW], px[:, 0:XW], OP.add)
                        if (j + pi) % 2 == 0:
                            nc.scalar.copy(XN2[:, XW:], px[:, XW:])
                        else:
                            nc.vector.tensor_copy(XN2[:, XW:], px[:, XW:])
                        XNs[pi] = XN2
                else:
                    for pi in P_:
                        px = ps.tile([128, XW], F32, tag="ps")
                        nc.tensor.matmul(px, npTs[pi], XNs[pi][:, 0:XW],
                                         start=True, stop=True)
                        Xfs[pi] = ck.tile([128, XW], F32, tag="Xf", bufs=9, name=f"Xf{pi}")
                        nc.vector.tensor_tensor(
                            Xfs[pi], XNs[pi][:, 0:XW], px, OP.add)

            for pi in P_:  # S7: chunk-local outputs + state maps
                pob = ps.tile([DV, 128], F32, tag="ps")
                nc.tensor.matmul(pob, Xfs[pi][:, 0:DV], attnTs[pi],
                                 start=True, stop=True)
                nc.scalar.copy(obS[:, pcs_[pi]], pob)
                # qeff in pair-blockdiag form: rows 0:32 even / 32:64 odd chunk
                pqb = ps.tile([64, 128], F32, tag="ps")
                nc.tensor.matmul(pqb[0:DKP, :], Xfs[pi][:, DV:], attnTs[pi],
                                 start=True, stop=True)
                nc.tensor.matmul(pqb[DKP:, :], Xfs[pi][:, DV:], attnTs[pi],
                                 start=True, stop=True, skip_group_check=True)
                qtmp = ck.tile([64, 128], F32, tag="qtmp", bufs=4,
                               name=f"qtmp{pi}")
                nc.vector.tensor_tensor(qtmp, pqb, m01S, OP.mult)
                nc.gpsimd.tensor_tensor(
                    qeS[:, pcs_[pi]], Qtbd[:, pcs_[pi]], qtmp, OP.subtract)
                pa2 = ps.tile([DKP, 2 * DKP], F32, tag="ps")
                nc.tensor.matmul(pa2, Xfs[pi][:, DV:], kbars[pi],
                                 start=True, stop=True)
                nc.vector.tensor_copy(
                    amS[:, 2 * DKP * pi:2 * DKP * pi + 2 * DKP], pa2)
                pbm = ps.tile([2 * DKP, DV], F32, tag="ps")
                nc.tensor.matmul(pbm, kbars[pi], Xfs[pi][:, 0:DV],
                                 start=True, stop=True)
                nc.scalar.copy(bmS[:, DV * pi:DV * pi + DV], pbm)

            nc.sync.dma_start(out=oqg[:, :], in_=oqgS)
            nc.sync.dma_start(out=scano[:, :], in_=scanS)
    nc.compile()
    return nc


# ---------------------------------------------------------------- L2 builder
Q_SE = TL
Q_PNW = Q_SE + NCH * DV
Q_WOUT = Q_PNW + 1
Q_ONES = Q_WOUT + D
C2W = Q_ONES + 128


def build_l2():
    nc = bacc.Bacc(None, target_bir_lowering=False)
    x = nc.dram_tensor("x", (TL, D), F32, kind="ExternalInput")
    xT = nc.dram_tensor("xT", (D, TL), BF16, kind="ExternalInput")
    # packed consts2: cols 0:TL rows[0:16 obase |16:32 gsig |32:64 qeff],
    # then sentry (rows 32:64), pnw, wout, ones
    consts2 = nc.dram_tensor("consts2", (128, C2W), F32, kind="ExternalInput")
    wout = nc.dram_tensor("wout", (DV, D), F32R, kind="ExternalInput")
    wg = nc.dram_tensor("wg", (D, D), F8, kind="ExternalInput")
    wu = nc.dram_tensor("wu", (D, D), F8, kind="ExternalInput")
    wd = nc.dram_tensor("wd", (D, D), F8, kind="ExternalInput")
    y = nc.dram_tensor("y", (TL, D), F32, kind="ExternalOutput")

    with tile.TileContext(nc) as tc:
        with (
            tc.tile_pool(name="big", bufs=1) as big,
            tc.tile_pool(name="work", bufs=3) as work,
            tc.tile_pool(name="psr", bufs=2, space="PSUM") as psr,
            tc.tile_pool(name="psb", bufs=1, space="PSUM") as psb,
            tc.tile_pool(name="psm", bufs=5, space="PSUM") as psm,
        ):
            cS2 = big.tile([128, C2W], F32)
            nc.sync.dma_start(out=cS2, in_=consts2[:, :])
            onesS = cS2[:, Q_ONES:Q_ONES + 128]
            obS = cS2[64:64 + DV, 0:TL]
            qeS = cS2[0:64, 0:TL]
            gsS = big.tile([DV, TL], F32)
            nc.sync.dma_start(out=gsS, in_=cS2[80:96, 0:TL])
            seS = cS2[0:64, Q_SE:Q_SE + NP * DV]
            woutS = big.tile([DV, D], F32R)
            nc.sync.dma_start(out=woutS, in_=wout[:, :])
            pnwS = cS2[0:DV, Q_PNW:Q_PNW + 1]
            epsS = big.tile([1, 1], F32)
            nc.vector.memset(epsS, EPS)
            x1b = big.tile([128, DT, TL], BF16)
            for hh in range(2):
                nc.sync.dma_start(
                    out=x1b[:, 4 * hh:4 * hh + 4, :],
                    in_=xT.rearrange("(j p) t -> p j t", p=128)
                    [:, 4 * hh:4 * hh + 4, :])
            x1s = [x1b[:, j, :] for j in range(DT)]
            # full-weight + x preloads (no deps -> overlap with o/norm phase)
            wgS = big.tile([128, DT, D], F8)
            nc.sync.dma_start(out=wgS, in_=wg.rearrange("(j p) f -> p j f", p=128))
            wuS = big.tile([128, DT, D], F8)
            nc.sync.dma_start(out=wuS, in_=wu.rearrange("(j p) f -> p j f", p=128))
            wds = big.tile([128, DT, D], F8)
            nc.sync.dma_start(out=wds, in_=wd.rearrange("(f p) d -> p f d", p=128))
            xbig = big.tile([128, TT, D], F32)
            nc.sync.dma_start(out=xbig, in_=x.rearrange("(i p) d -> p i d", p=128))

            # ---- o^T assembly (blockdiag sentry, one matmul per pair) ----
            oT = big.tile([DV, TL], F32)
            for pi in range(NP):
                pcs = slice(128 * pi, 128 * pi + 128)
                po = psr.tile([DV, 128], F32, tag="red")
                nc.tensor.matmul(po, seS[:, DV * pi:DV * pi + DV], qeS[:, pcs],
                                 start=True, stop=True)
                nc.vector.tensor_tensor(oT[:, pcs], obS[:, pcs], po, OP.add)

            # ---- post rmsnorm + gate:  og = rms(o)*pnw*gsig ----
            osq = big.tile([DV, TL], F32)
            nc.scalar.activation(osq, oT, AF.Square)
            og = big.tile([DV, TL], F32R)
            for n in range(2):
                sl = slice(512 * n, 512 * n + 512)
                prs = psr.tile([1, 512], F32, tag="red")
                nc.tensor.matmul(prs, onesS[0:DV, 0:1], osq[:, sl],
                                 start=True, stop=True)
                rq = work.tile([1, 512], F32, tag="rq")
                nc.scalar.activation(rq, prs, AF.Sqrt, scale=1.0 / DV, bias=epsS[:, :])
                rr = work.tile([1, 512], F32, tag="rr")
                nc.vector.reciprocal(rr, rq)
                pbv = psr.tile([DV, 512], F32, tag="red")
                nc.tensor.matmul(pbv, onesS[0:1, 0:DV], rr,
                                 start=True, stop=True)
                t1 = work.tile([DV, 512], F32, tag="t1")
                nc.vector.tensor_tensor(t1, oT[:, sl], pbv, OP.mult)
                t2 = work.tile([DV, 512], F32, tag="t2")
                nc.vector.tensor_scalar(t2, t1, pnwS[:, :], None, OP.mult)
                nc.vector.tensor_tensor(og[:, sl], t2, gsS[:, sl], OP.mult)

            # ---- x1 = x^T + Wout^T og -> ffn rmsnorm -> h8, streamed per
            # token-slice n so slice-0 FFN overlaps slice-1 normalization
            h8 = big.tile([128, DT, TL], F8)
            for n in range(2):
                sl = slice(512 * n, 512 * n + 512)
                for j in range(DT):
                    px1 = psm.tile([128, 512], F32, tag="mm")
                    nc.tensor.matmul(px1, woutS[:, 128 * j:128 * j + 128],
                                     og[:, sl], start=True, stop=True)
                    nc.vector.tensor_tensor(x1s[j][:, sl], x1s[j][:, sl], px1,
                                            OP.add)
                ph = psr.tile([1, 512], F32, tag="red")
                for j in range(DT):
                    sq = work.tile([128, 512], F32, tag="sq")
                    nc.scalar.activation(sq, x1s[j][:, sl], AF.Square)
                    nc.tensor.matmul(ph, onesS[:, 0:1], sq,
                                     start=(j == 0), stop=(j == DT - 1))
                r1q = work.tile([1, 512], F32, tag="r1q")
                nc.scalar.activation(r1q, ph, AF.Sqrt, scale=1.0 / D, bias=epsS[:, :])
                r1 = work.tile([1, 512], F32, tag="r1")
                nc.vector.reciprocal(r1, r1q)
                pbb = psb.tile([128, 512], F32, tag="bcb")
                nc.tensor.matmul(pbb, onesS[0:1, :], r1,
                                 start=True, stop=True)
                rbn = big.tile([128, 512], F32, name=f"rb{n}")
                nc.scalar.copy(rbn, pbb)
                for j in range(DT):
                    eng = nc.gpsimd if (2 * j + n) % 3 == 2 else nc.vector
                    eng.tensor_tensor(h8[:, j, sl], x1s[j][:, sl], rbn, OP.mult)

            # ---- gate/up -> z (fp8 DoubleRow matmuls); n-outer so the down
            # projection for token-slice n can overlap gate/up of slice n+1
            zs8 = big.tile([128, DT, TL], F8)
            for n in range(2):
                sl = slice(512 * n, 512 * n + 512)
                for f in range(DT):
                    fs = slice(128 * f, 128 * f + 128)
                    pg = psm.tile([128, 512], F32, tag="mm")
                    for j in range(DT // 2):
                        nc.tensor.matmul(pg, wgS[:, 2 * j:2 * j + 2, fs],
                                         h8[:, 2 * j:2 * j + 2, sl],
                                         start=(j == 0), stop=(j == DT // 2 - 1),
                                         perf_mode=DR)
                    pu = psm.tile([128, 512], F32, tag="mm")
                    for j in range(DT // 2):
                        nc.tensor.matmul(pu, wuS[:, 2 * j:2 * j + 2, fs],
                                         h8[:, 2 * j:2 * j + 2, sl],
                                         start=(j == 0), stop=(j == DT // 2 - 1),
                                         perf_mode=DR)
                    gs = work.tile([128, 512], F32, tag="gs")
                    nc.scalar.activation(gs, pg, AF.Silu)
                    nc.vector.tensor_tensor(zs8[:, f, sl], pu, gs, OP.mult)

                # ---- down + wout-term + x residual -> y (tiles of slice n) ----
                for i in range(4 * n, 4 * n + 4):
                    ts = slice(128 * i, 128 * i + 128)
                    for n2 in range(2):
                        sl2 = slice(512 * n2, 512 * n2 + 512)
                        pd = psm.tile([128, 512], F32, tag="mm")
                        for f in range(DT // 2):
                            nc.tensor.matmul(pd, zs8[:, 2 * f:2 * f + 2, ts],
                                             wds[:, 2 * f:2 * f + 2, sl2],
                                             start=(f == 0), stop=False,
                                             perf_mode=DR)
                        nc.tensor.matmul(pd, og[:, ts], woutS[:, sl2],
                                         start=False, stop=True,
                                         skip_group_check=True)
                        yo = work.tile([128, 512], F32, tag="yo")
                        nc.vector.tensor_tensor(yo, pd, xbig[:, i, sl2], OP.add)
                        nc.sync.dma_start(out=y[ts, sl2], in_=yo)
    nc.compile()
    return nc


# ---------------------------------------------------------------- host glue
_CACHE = {}
LAST_RESULTS = {}  # populated with BassKernelResults per launch (for test.py)


def kernel(**inputs):
    x_seq = np.ascontiguousarray(np.asarray(inputs["x_seq"], np.float32))

    def sigmoid(z):
        return 1.0 / (1.0 + np.exp(-z))

    positions = np.arange(T, dtype=np.float32)
    freqs = THETA ** (np.arange(DK, dtype=np.float32) / DK)
    phi = positions[:, None] * freqs[None, :]
    psi = 2.0 * math.pi * sigmoid(np.asarray(inputs["pope_delta_raw"], np.float32))
    trig_full = -np.concatenate(
        [np.cos(phi).T, np.sin(phi).T, np.cos(phi - psi).T, np.sin(phi - psi).T],
        axis=0).astype(np.float32)  # (64, T); negated: device mu = -softplus
    # blockdiag-masked trig: [q*even; q*odd; k*even; k*odd] per chunk parity
    par = ((positions.astype(np.int64) // C) % 2).astype(np.float32)  # (T,)
    evm, odm = (1.0 - par)[None, :], par[None, :]
    trig_bd = np.concatenate(
        [trig_full[0:DKP] * evm, trig_full[0:DKP] * odm,
         trig_full[DKP:] * evm, trig_full[DKP:] * odm], axis=0).astype(np.float32)
    wall = np.zeros((D, NW), np.float32)
    wall[:, 0:16] = np.asarray(inputs["Wq"], np.float32)
    wall[:, 16:32] = np.asarray(inputs["Wk"], np.float32)
    wall[:, 32:64] = np.asarray(inputs["Wa1"], np.float32)
    wall[:, 64:80] = np.asarray(inputs["Wv"], np.float32)
    wall[:, 96:97] = np.asarray(inputs["Wbeta"], np.float32)
    wall[:, 97:113] = np.asarray(inputs["Wgate"], np.float32)
    i128 = np.eye(128, dtype=np.float32)
    permqk = np.zeros((DKP, 2 * DKP), np.float32)
    for r in range(DKP):
        permqk[r % DK, r] = 1.0          # qrep: out row r <- qmu row r%16
        permqk[DK + r % DK, DKP + r] = 1.0  # krep
    permbd = np.zeros((DKP, 128), np.float32)
    for cc in range(128):
        r = cc % DKP
        permbd[(r % DK) if cc < 64 else (DK + r % DK), cc] = 1.0

    mskP = np.zeros((128, 128), np.float32)
    nmskP = np.zeros((128, 128), np.float32)
    for b in range(2):
        bs = slice(C * b, C * b + C)
        mskP[bs, bs] = np.triu(np.ones((C, C), np.float32), 0)
        nmskP[bs, bs] = np.tril(-np.ones((C, C), np.float32), -1)

    if "l1" not in _CACHE:
        _CACHE["l1"] = build_l1()
    bf16 = mybir.dt.np(BF16)
    x16 = x_seq.astype(bf16)
    # bf16 data reinterpreted as f32 words for the packed consts tensor
    wallw = np.ascontiguousarray(
        wall.astype(bf16).reshape(DT, 128, NW).transpose(1, 0, 2)
        .reshape(128, DT * NW)).view(np.float32)
    i16w = np.ascontiguousarray(i128.astype(bf16)).view(np.float32)
    in1 = []
    for m in range(NCORE):
        sl = slice(TL * m, TL * m + TL)
        cs = np.zeros((128, CW), np.float32)
        cs[:, O_ID:O_ID + 128] = i128
        cs[:, O_MSK:O_MSK + 128] = mskP
        cs[:, O_NMSK:O_NMSK + 128] = nmskP
        cs[:, O_TBD:O_TBD + TL] = trig_bd[:, sl]
        cs[0:64, O_TRG:O_TRG + TL] = trig_full[:, sl]
        cs[:, O_WALL:O_WALL + 512] = wallw.view(np.float32)
        cs[:, O_I16:O_I16 + 64] = i16w.view(np.float32)
        cs[0:DKP, O_WA2:O_WA2 + DKP] = np.asarray(inputs["Wa2"], np.float32)
        cs[0:DKP, O_PQK:O_PQK + 2 * DKP] = permqk
        cs[0:DKP, O_PBD:O_PBD + 128] = permbd
        cs[0:DKP, O_M01:O_M01 + 64] = 1.0
        cs[DKP:64, O_M01 + 64:O_M01 + 128] = 1.0
        in1.append({"x": np.ascontiguousarray(x16[sl]), "consts": cs})
    r1 = run_bass_kernel_spmd(_CACHE["l1"], in1, core_ids=list(range(NCORE)))
    LAST_RESULTS["l1"] = r1
    res1 = r1.results

    # host chunk-state scan (128 tiny steps); sentry in pair-blockdiag rows
    S = np.zeros((DKP, DV), np.float32)
    sentries = []
    for m in range(NCORE):
        se = np.zeros((64, NP * DV), np.float32)
        sc = res1[m]["scano"]
        am, bm = sc[0:DKP, 0:512], sc[:, 512:512 + NP * DV]
        pc = sc[0:DKP, 640:640 + NCH]
        for c in range(NCH):
            p, odd = c // 2, c % 2
            se[DKP * odd:DKP * odd + DKP, DV * p:DV * p + DV] = S
            wtk = am[:, 2 * DKP * p + DKP * odd:2 * DKP * p + DKP * odd + DKP]
            AT = np.diag(pc[:, c]) - wtk
            B = bm[DKP * odd:DKP * odd + DKP, DV * p:DV * p + DV]
            S = AT.T @ S + B
        sentries.append(se)

    f8np = mybir.dt.np(F8)
    ffnw = np.asarray(inputs["ffn_norm_w"], np.float32)[:, None]
    wgm = np.ascontiguousarray(
        (ffnw * np.asarray(inputs["Wffn_gate"], np.float32)).astype(f8np))
    wum = np.ascontiguousarray(
        (ffnw * np.asarray(inputs["Wffn_up"], np.float32)).astype(f8np))
    wdm = np.ascontiguousarray(
        np.asarray(inputs["Wffn_down"], np.float32).astype(f8np))
    onesc = np.ones((128, 128), np.float32)

    if "l2" not in _CACHE:
        _CACHE["l2"] = build_l2()
    in2 = []
    for m in range(NCORE):
        sl = slice(TL * m, TL * m + TL)
        c2 = np.zeros((128, C2W), np.float32)
        c2[:, 0:TL] = res1[m]["oqg"]
        c2[0:64, Q_SE:Q_SE + NP * DV] = sentries[m]
        c2[0:DV, Q_PNW] = np.asarray(inputs["post_norm_w"], np.float32)
        c2[:, Q_ONES:Q_ONES + 128] = onesc
        in2.append({
            "x": np.ascontiguousarray(x_seq[sl]),
            "xT": res1[m]["xT"], "consts2": c2,
            "wout": np.ascontiguousarray(inputs["Wout"]),
            "wg": wgm, "wu": wum, "wd": wdm,
        })
    r2 = run_bass_kernel_spmd(_CACHE["l2"], in2, core_ids=list(range(NCORE)))
    LAST_RESULTS["l2"] = r2
    res2 = r2.results
    return np.concatenate([res2[m]["y"] for m in range(NCORE)], axis=0)

